# revision 2
# baseline (speedup 1.0000x reference)
"""nn_GateMulti — MoE routing (8 experts, one-hot gate) on 8 TRN2 NeuronCores.

Strategy: expert-parallel. The gate is exactly one-hot on groups[:, 0], so
each token needs exactly one expert's MLP. Host-side "all-to-all": sort the
4096 tokens by expert id, pad each expert's token set to a common capacity,
and hand core e exactly expert e's tokens (transposed) plus expert e's
weights. Each core then runs a dense 2-layer MLP:

    yT = W2.T @ relu(W1.T @ xT + b1) + b2        (feature-major layout)

with float32r (FP22-truncated fp32) matmuls, which stream at full PE rate
for moving dims >= 256. The host scatters per-core outputs back to the
original token order. Compute per core is ~cap/512 of the ideal balanced
load (cap = max tokens routed to one expert), ~8x less than the dense
all-experts reference.

Self-contained: shapes hardcoded from the problem spec.
"""

import math
from functools import lru_cache

import numpy as np

import concourse.bacc as bacc
import concourse.mybir as mybir
import concourse.tile as tile
from concourse.bass_utils import run_bass_kernel_spmd

E = 8
B = 4096
D_IN = 512
D_FF = 2048
D_OUT = 512
GROUP_COL = 0

P = 128
D_T = D_IN // P   # 4  k-tiles for layer 1
F_T = D_FF // P   # 16 f-tiles (layer-1 out / layer-2 contraction)
O_T = D_OUT // P  # 4  o-tiles for layer 2

F32 = mybir.dt.float32
F32R = mybir.dt.float32r


def _emit(tc, nc, xT, w1, w2, b1t, b2t, yT, cap, n_chunks, chunk):
    relu = mybir.ActivationFunctionType.Relu
    ident = mybir.ActivationFunctionType.Identity

    with (
        tc.tile_pool(name="consts", bufs=1) as cpool,
        tc.tile_pool(name="acts", bufs=1) as apool,
        tc.tile_pool(name="yout", bufs=4) as ypool,
        tc.tile_pool(name="psum_h", bufs=4, space="PSUM") as ph,
        tc.tile_pool(name="psum_y", bufs=4, space="PSUM") as py,
    ):
        # ---- input DMAs, emitted in the order compute consumes them ----
        xT_sb = apool.tile([P, D_T, cap], F32R)
        xT_r = xT.ap().rearrange("(j p) c -> p j c", p=P)
        for c in range(n_chunks):
            cs = slice(c * chunk, (c + 1) * chunk)
            nc.sync.dma_start(xT_sb[:, :, cs], xT_r[:, :, cs])

        b1_sb = cpool.tile([P, F_T], F32)
        nc.sync.dma_start(b1_sb[:], b1t.ap())
        b2_sb = cpool.tile([P, O_T], F32)
        nc.sync.dma_start(b2_sb[:], b2t.ap())

        # W1[d, f] -> sbuf [p, j, i, 128] so lhsT tile (j, i) is W1 block
        w1_sb = cpool.tile([P, D_T, F_T, P], F32R)
        w1_r = w1.ap().rearrange("(j p) (i c) -> p j i c", p=P, c=P)
        for i in range(F_T):
            nc.sync.dma_start(w1_sb[:, :, i, :], w1_r[:, :, i, :])

        # W2[f, o] -> sbuf [p, i, k, 128]
        w2_sb = cpool.tile([P, F_T, O_T, P], F32R)
        w2_r = w2.ap().rearrange("(i p) (k c) -> p i k c", p=P, c=P)
        for k in range(O_T):
            nc.sync.dma_start(w2_sb[:, :, k, :], w2_r[:, :, k, :])

        hT_sb = apool.tile([P, F_T, cap], F32R)

        for c in range(n_chunks):
            cs = slice(c * chunk, (c + 1) * chunk)
            # ---- layer 1: hT[f, c] = relu(sum_d W1[d, f] xT[d, c] + b1[f])
            for i in range(F_T):
                hp = ph.tile([P, chunk], F32, name=f"hp_{c}_{i}", tag="hp")
                for j in range(D_T):
                    nc.tensor.matmul(
                        hp[:],
                        w1_sb[:, j, i, :],
                        xT_sb[:, j, cs],
                        start=(j == 0),
                        stop=(j == D_T - 1),
                    )
                nc.scalar.activation(
                    hT_sb[:, i, cs], hp[:], relu, bias=b1_sb[:, i : i + 1]
                )
            # ---- layer 2: yT[o, c] = sum_f W2[f, o] hT[f, c] + b2[o]
            for k in range(O_T):
                yp = py.tile([P, chunk], F32, name=f"yp_{c}_{k}", tag="yp")
                for i in range(F_T):
                    nc.tensor.matmul(
                        yp[:],
                        w2_sb[:, i, k, :],
                        hT_sb[:, i, cs],
                        start=(i == 0),
                        stop=(i == F_T - 1),
                    )
                yo = ypool.tile([P, chunk], F32, name=f"yo_{c}_{k}", tag="yo")
                nc.scalar.activation(yo[:], yp[:], ident, bias=b2_sb[:, k : k + 1])
                nc.sync.dma_start(yT[k * P : (k + 1) * P, cs], yo[:])


@lru_cache(maxsize=4)
def _build_nc(cap, n_chunks, chunk):
    nc = bacc.Bacc("TRN2", target_bir_lowering=False, debug=False, num_devices=E)
    xT = nc.dram_tensor("xT", [D_IN, cap], F32R, kind="ExternalInput")
    w1 = nc.dram_tensor("w1", [D_IN, D_FF], F32R, kind="ExternalInput")
    w2 = nc.dram_tensor("w2", [D_FF, D_OUT], F32R, kind="ExternalInput")
    b1t = nc.dram_tensor("b1t", [P, F_T], F32, kind="ExternalInput")
    b2t = nc.dram_tensor("b2t", [P, O_T], F32, kind="ExternalInput")
    yT = nc.dram_tensor("yT", [D_OUT, cap], F32, kind="ExternalOutput")
    with tile.TileContext(nc) as tc:
        _emit(tc, nc, xT, w1, w2, b1t, b2t, yT, cap, n_chunks, chunk)
    nc.compile()
    return nc


def _plan_capacity(max_count):
    cap0 = max(int(max_count), 16)
    n_chunks = max(1, math.ceil(cap0 / 512))
    chunk = math.ceil(cap0 / (n_chunks * 16)) * 16
    return n_chunks * chunk, n_chunks, chunk


def _shard(x, groups, W1, b1, W2, b2):
    idx = np.asarray(groups)[:, GROUP_COL].astype(np.int64)
    order = np.argsort(idx, kind="stable")
    counts = np.bincount(idx, minlength=E)
    cap, n_chunks, chunk = _plan_capacity(counts.max())
    offs = np.concatenate([[0], np.cumsum(counts)])

    x = np.asarray(x, dtype=np.float32)
    W1 = np.asarray(W1, dtype=np.float32)
    b1 = np.asarray(b1, dtype=np.float32)
    W2 = np.asarray(W2, dtype=np.float32)
    b2 = np.asarray(b2, dtype=np.float32)

    in_maps, tok_ids = [], []
    for e in range(E):
        ids = order[offs[e] : offs[e + 1]]
        tok_ids.append(ids)
        xT = np.zeros((D_IN, cap), np.float32)
        xT[:, : len(ids)] = x[ids].T
        in_maps.append(
            {
                "xT": xT,
                "w1": np.ascontiguousarray(W1[e]),
                "w2": np.ascontiguousarray(W2[e]),
                "b1t": np.ascontiguousarray(b1[e].reshape(F_T, P).T),
                "b2t": np.ascontiguousarray(b2[e].reshape(O_T, P).T),
            }
        )
    return in_maps, tok_ids, counts, cap, n_chunks, chunk


def _run(x, groups, W1, b1, W2, b2, trace=False, **spmd_kwargs):
    in_maps, tok_ids, counts, cap, n_chunks, chunk = _shard(x, groups, W1, b1, W2, b2)
    nc = _build_nc(cap, n_chunks, chunk)
    res = run_bass_kernel_spmd(
        nc, in_maps, core_ids=list(range(E)), trace=trace, **spmd_kwargs
    )
    out = np.zeros((B, D_OUT), np.float32)
    for e in range(E):
        yTe = res.results[e]["yT"]
        out[tok_ids[e]] = yTe[:, : counts[e]].T
    return out, res


def kernel(x, groups, W1, b1, W2, b2):
    out, _ = _run(x, groups, W1, b1, W2, b2)
    return out


# revision 4
# speedup vs baseline: 1.1231x; 1.1231x over previous
"""nn_GateMulti — MoE routing (8 experts, one-hot gate) on 8 TRN2 NeuronCores.

Strategy: expert-parallel. The gate is exactly one-hot on groups[:, 0], so
each token needs exactly one expert's MLP. Host-side "all-to-all": sort the
4096 tokens by expert id, pad each expert's token set to a common capacity,
and hand core e exactly expert e's tokens (transposed) plus expert e's
weights. Each core then runs a dense 2-layer MLP:

    yT = W2.T @ relu(W1.T @ xT + b1) + b2        (feature-major layout)

Moving operands (xT, hT) are float32r (FP22-truncated fp32) which streams at
full PE rate for moving dims >= 256; stationary weights are bf16 (halves
weight DMA, enables fast-weight-load). The host scatters per-core outputs
back to the original token order. Compute per core is ~cap/512 of the ideal
balanced load (cap = max tokens routed to one expert), ~8x less than the
dense all-experts reference.

Weights are host-packed into the exact SBUF image layout so every weight DMA
moves multi-KB contiguous lines; w1 streams on the sync HWDGE ring while w2
streams on the scalar HWDGE ring.

Self-contained: shapes hardcoded from the problem spec.
"""

import math
from functools import lru_cache

import ml_dtypes
import numpy as np

import concourse.bacc as bacc
import concourse.mybir as mybir
import concourse.tile as tile
from concourse.bass_utils import run_bass_kernel_spmd

E = 8
B = 4096
D_IN = 512
D_FF = 2048
D_OUT = 512
GROUP_COL = 0

P = 128
D_T = D_IN // P   # 4  k-tiles for layer 1
F_T = D_FF // P   # 16 f-tiles (layer-1 out / layer-2 contraction)
O_T = D_OUT // P  # 4  o-tiles for layer 2
W1_G = 4          # w1 DMA granularity: F_T/W1_G f-tiles per DMA

F32 = mybir.dt.float32
F32R = mybir.dt.float32r
BF16 = mybir.dt.bfloat16

# walrus requires matmul operand dtypes to match when fp32/fp32r is involved,
# so weights and moving activations share one dtype: bf16 (fast weight load,
# half the DMA bytes) or float32r (better precision, full PE rate at N>=256).
ALL_BF16 = True
W_DT = A_DT = BF16 if ALL_BF16 else F32R
W_NP = ml_dtypes.bfloat16 if ALL_BF16 else np.float32


def _emit(tc, nc, xT, w1, w2, b1t, b2t, yT, cap, n_chunks, chunk):
    relu = mybir.ActivationFunctionType.Relu
    ident = mybir.ActivationFunctionType.Identity

    with (
        tc.tile_pool(name="consts", bufs=1) as cpool,
        tc.tile_pool(name="acts", bufs=1) as apool,
        tc.tile_pool(name="yout", bufs=4) as ypool,
        tc.tile_pool(name="psum_h", bufs=4, space="PSUM") as ph,
        tc.tile_pool(name="psum_y", bufs=4, space="PSUM") as py,
    ):
        # ---- input DMAs, emitted in the order compute consumes them.
        # w1 + xT + biases stream on the sync HWDGE ring; w2 on the scalar
        # HWDGE ring so both rings pull from HBM concurrently.
        w1_sb = cpool.tile([P, F_T, D_T, P], W_DT)   # [p, i, j, c]
        w2_sb = cpool.tile([P, O_T, F_T, P], W_DT)   # [p, k, i, c]
        xT_sb = apool.tile([P, D_T, cap], A_DT)
        xT_r = xT.ap().rearrange("(j p) c -> p j c", p=P)

        ng = F_T // W1_G
        nc.sync.dma_start(w1_sb[:, 0:W1_G], w1.ap()[:, 0:W1_G])
        for k in range(O_T):
            nc.scalar.dma_start(w2_sb[:, k], w2.ap()[:, k])
        nc.sync.dma_start(xT_sb[:, :, 0:chunk], xT_r[:, :, 0:chunk])
        b1_sb = cpool.tile([P, F_T], F32)
        nc.sync.dma_start(b1_sb[:], b1t.ap())
        for g in range(1, ng):
            nc.sync.dma_start(
                w1_sb[:, g * W1_G : (g + 1) * W1_G], w1.ap()[:, g * W1_G : (g + 1) * W1_G]
            )
        for c in range(1, n_chunks):
            cs = slice(c * chunk, (c + 1) * chunk)
            nc.sync.dma_start(xT_sb[:, :, cs], xT_r[:, :, cs])
        b2_sb = cpool.tile([P, O_T], F32)
        nc.sync.dma_start(b2_sb[:], b2t.ap())

        hT_sb = apool.tile([P, F_T, cap], A_DT)

        for c in range(n_chunks):
            cs = slice(c * chunk, (c + 1) * chunk)
            # ---- layer 1: hT[f, c] = relu(sum_d W1[d, f] xT[d, c] + b1[f])
            for i in range(F_T):
                hp = ph.tile([P, chunk], F32, name=f"hp_{c}_{i}", tag="hp")
                for j in range(D_T):
                    nc.tensor.matmul(
                        hp[:],
                        w1_sb[:, i, j, :],
                        xT_sb[:, j, cs],
                        start=(j == 0),
                        stop=(j == D_T - 1),
                    )
                nc.scalar.activation(
                    hT_sb[:, i, cs], hp[:], relu, bias=b1_sb[:, i : i + 1]
                )
            # ---- layer 2: yT[o, c] = sum_f W2[f, o] hT[f, c] + b2[o]
            for k in range(O_T):
                yp = py.tile([P, chunk], F32, name=f"yp_{c}_{k}", tag="yp")
                for i in range(F_T):
                    nc.tensor.matmul(
                        yp[:],
                        w2_sb[:, k, i, :],
                        hT_sb[:, i, cs],
                        start=(i == 0),
                        stop=(i == F_T - 1),
                    )
                yo = ypool.tile([P, chunk], F32, name=f"yo_{c}_{k}", tag="yo")
                nc.scalar.activation(yo[:], yp[:], ident, bias=b2_sb[:, k : k + 1])
                nc.sync.dma_start(yT[k * P : (k + 1) * P, cs], yo[:])


@lru_cache(maxsize=4)
def _build_nc(cap, n_chunks, chunk):
    nc = bacc.Bacc("TRN2", target_bir_lowering=False, debug=False, num_devices=E)
    xT = nc.dram_tensor("xT", [D_IN, cap], A_DT, kind="ExternalInput")
    w1 = nc.dram_tensor("w1", [P, F_T, D_T, P], W_DT, kind="ExternalInput")
    w2 = nc.dram_tensor("w2", [P, O_T, F_T, P], W_DT, kind="ExternalInput")
    b1t = nc.dram_tensor("b1t", [P, F_T], F32, kind="ExternalInput")
    b2t = nc.dram_tensor("b2t", [P, O_T], F32, kind="ExternalInput")
    yT = nc.dram_tensor("yT", [D_OUT, cap], F32, kind="ExternalOutput")
    with tile.TileContext(nc) as tc:
        _emit(tc, nc, xT, w1, w2, b1t, b2t, yT, cap, n_chunks, chunk)
    nc.compile()
    return nc


def _plan_capacity(max_count):
    cap0 = max(int(max_count), 16)
    n_chunks = max(1, math.ceil(cap0 / 512))
    chunk = math.ceil(cap0 / (n_chunks * 16)) * 16
    return n_chunks * chunk, n_chunks, chunk


def _pack_w1(W1e):
    # w1img[p, i, j, c] = W1e[j*128 + p, i*128 + c]
    return np.ascontiguousarray(
        W1e.reshape(D_T, P, F_T, P).transpose(1, 2, 0, 3).astype(W_NP)
    )


def _pack_w2(W2e):
    # w2img[p, k, i, c] = W2e[i*128 + p, k*128 + c]
    return np.ascontiguousarray(
        W2e.reshape(F_T, P, O_T, P).transpose(1, 2, 0, 3).astype(W_NP)
    )


def _shard(x, groups, W1, b1, W2, b2):
    idx = np.asarray(groups)[:, GROUP_COL].astype(np.int64)
    order = np.argsort(idx, kind="stable")
    counts = np.bincount(idx, minlength=E)
    cap, n_chunks, chunk = _plan_capacity(counts.max())
    offs = np.concatenate([[0], np.cumsum(counts)])

    x = np.asarray(x, dtype=np.float32)
    W1 = np.asarray(W1, dtype=np.float32)
    b1 = np.asarray(b1, dtype=np.float32)
    W2 = np.asarray(W2, dtype=np.float32)
    b2 = np.asarray(b2, dtype=np.float32)

    in_maps, tok_ids = [], []
    for e in range(E):
        ids = order[offs[e] : offs[e + 1]]
        tok_ids.append(ids)
        xT = np.zeros((D_IN, cap), np.float32)
        xT[:, : len(ids)] = x[ids].T
        xT = xT.astype(W_NP)
        in_maps.append(
            {
                "xT": xT,
                "w1": _pack_w1(W1[e]),
                "w2": _pack_w2(W2[e]),
                "b1t": np.ascontiguousarray(b1[e].reshape(F_T, P).T),
                "b2t": np.ascontiguousarray(b2[e].reshape(O_T, P).T),
            }
        )
    return in_maps, tok_ids, counts, cap, n_chunks, chunk


def _run(x, groups, W1, b1, W2, b2, trace=False, **spmd_kwargs):
    in_maps, tok_ids, counts, cap, n_chunks, chunk = _shard(x, groups, W1, b1, W2, b2)
    nc = _build_nc(cap, n_chunks, chunk)
    res = run_bass_kernel_spmd(
        nc, in_maps, core_ids=list(range(E)), trace=trace, **spmd_kwargs
    )
    out = np.zeros((B, D_OUT), np.float32)
    for e in range(E):
        yTe = res.results[e]["yT"]
        out[tok_ids[e]] = yTe[:, : counts[e]].T
    return out, res


def kernel(x, groups, W1, b1, W2, b2):
    out, _ = _run(x, groups, W1, b1, W2, b2)
    return out


# revision 6
# speedup vs baseline: 1.1434x; 1.0182x over previous
"""nn_GateMulti — MoE routing (8 experts, one-hot gate) on 8 TRN2 NeuronCores.

Strategy: expert-parallel. The gate is exactly one-hot on groups[:, 0], so
each token needs exactly one expert's MLP. Host-side "all-to-all": sort the
4096 tokens by expert id, pad each expert's token set to a common capacity,
and hand core e exactly expert e's tokens (transposed) plus expert e's
weights. Each core then runs a dense 2-layer MLP:

    yT = W2.T @ relu(W1.T @ xT + b1) + b2        (feature-major layout)

Moving operands (xT, hT) are float32r (FP22-truncated fp32) which streams at
full PE rate for moving dims >= 256; stationary weights are bf16 (halves
weight DMA, enables fast-weight-load). The host scatters per-core outputs
back to the original token order. Compute per core is ~cap/512 of the ideal
balanced load (cap = max tokens routed to one expert), ~8x less than the
dense all-experts reference.

Weights are host-packed into the exact SBUF image layout so every weight DMA
moves multi-KB contiguous lines; w1 streams on the sync HWDGE ring while w2
streams on the scalar HWDGE ring.

Self-contained: shapes hardcoded from the problem spec.
"""

import math
from functools import lru_cache

import ml_dtypes
import numpy as np

import concourse.bacc as bacc
import concourse.mybir as mybir
import concourse.tile as tile
from concourse.bass_utils import run_bass_kernel_spmd

E = 8
B = 4096
D_IN = 512
D_FF = 2048
D_OUT = 512
GROUP_COL = 0

P = 128
D_T = D_IN // P   # 4  k-tiles for layer 1
F_T = D_FF // P   # 16 f-tiles (layer-1 out / layer-2 contraction)
O_T = D_OUT // P  # 4  o-tiles for layer 2
W1_G = 4          # w1 DMA granularity: F_T/W1_G f-tiles per DMA (g0 = pilot)

F32 = mybir.dt.float32
F32R = mybir.dt.float32r
BF16 = mybir.dt.bfloat16

# walrus requires matmul operand dtypes to match when fp32/fp32r is involved,
# so weights and moving activations share one dtype: bf16 (fast weight load,
# half the DMA bytes) or float32r (better precision, full PE rate at N>=256).
ALL_BF16 = True
W_DT = A_DT = BF16 if ALL_BF16 else F32R
W_NP = ml_dtypes.bfloat16 if ALL_BF16 else np.float32


def _emit(tc, nc, xT, w1, w2, b1t, b2t, yT, cap, n_chunks, chunk):
    relu = mybir.ActivationFunctionType.Relu
    ident = mybir.ActivationFunctionType.Identity
    from concourse.bass import _add_dep_helper

    with (
        tc.tile_pool(name="consts", bufs=1) as cpool,
        tc.tile_pool(name="acts", bufs=1) as apool,
        tc.tile_pool(name="yout", bufs=4) as ypool,
        tc.tile_pool(name="psum_h", bufs=4, space="PSUM") as ph,
        tc.tile_pool(name="psum_y", bufs=4, space="PSUM") as py,
    ):
        # ---- pilot DMAs: the minimum needed to start the PE (first w1
        # i-slices, all of xT, b1). They get the HWDGE rings exclusively --
        # everything else is gated behind the first matmul, because the DMA
        # engines fair-share packets across all outstanding transfers and
        # bulk weight traffic would starve these critical first bytes.
        w1_sb = cpool.tile([P, F_T, D_T, P], W_DT)   # [p, i, j, c]
        w2_sb = cpool.tile([P, O_T, F_T, P], W_DT)   # [p, k, i, c]
        xT_sb = apool.tile([P, D_T, cap], A_DT)
        xT_r = xT.ap().rearrange("(j p) c -> p j c", p=P)

        nc.sync.dma_start(w1_sb[:, 0:W1_G], w1.ap()[:, 0:W1_G])
        nc.scalar.dma_start(xT_sb[:], xT_r)
        b1_sb = cpool.tile([P, F_T], F32)
        nc.sync.dma_start(b1_sb[:], b1t.ap())

        # ---- bulk DMAs (issued on both rings, released once PE starts)
        bulk = []
        ng = F_T // W1_G
        for g in range(1, ng):
            bulk.append(
                nc.sync.dma_start(
                    w1_sb[:, g * W1_G : (g + 1) * W1_G],
                    w1.ap()[:, g * W1_G : (g + 1) * W1_G],
                )
            )
        for k in range(O_T):
            bulk.append(nc.scalar.dma_start(w2_sb[:, k], w2.ap()[:, k]))
        b2_sb = cpool.tile([P, O_T], F32)
        bulk.append(nc.scalar.dma_start(b2_sb[:], b2t.ap()))

        hT_sb = apool.tile([P, F_T, cap], A_DT)

        first_mm = None
        # ---- layer 1: hT[f, c] = relu(sum_d W1[d, f] xT[d, c] + b1[f])
        # chunk-interleaved so each w1 i-slice is consumed over 2x the time
        for i in range(F_T):
            for c in range(n_chunks):
                cs = slice(c * chunk, (c + 1) * chunk)
                hp = ph.tile([P, chunk], F32, name=f"hp_{i}_{c}", tag="hp")
                for j in range(D_T):
                    mm = nc.tensor.matmul(
                        hp[:],
                        w1_sb[:, i, j, :],
                        xT_sb[:, j, cs],
                        start=(j == 0),
                        stop=(j == D_T - 1),
                    )
                    if first_mm is None:
                        first_mm = mm
                nc.scalar.activation(
                    hT_sb[:, i, cs], hp[:], relu, bias=b1_sb[:, i : i + 1]
                )
        for d in bulk:
            _add_dep_helper(d.ins, first_mm.ins, sync=True, reason="hold bulk DMA until pilot set landed")
        # ---- layer 2: yT[o, c] = sum_f W2[f, o] hT[f, c] + b2[o]
        for k in range(O_T):
            for c in range(n_chunks):
                cs = slice(c * chunk, (c + 1) * chunk)
                yp = py.tile([P, chunk], F32, name=f"yp_{k}_{c}", tag="yp")
                for i in range(F_T):
                    nc.tensor.matmul(
                        yp[:],
                        w2_sb[:, k, i, :],
                        hT_sb[:, i, cs],
                        start=(i == 0),
                        stop=(i == F_T - 1),
                    )
                yo = ypool.tile([P, chunk], F32, name=f"yo_{k}_{c}", tag="yo")
                nc.scalar.activation(yo[:], yp[:], ident, bias=b2_sb[:, k : k + 1])
                nc.sync.dma_start(yT[k * P : (k + 1) * P, cs], yo[:])


@lru_cache(maxsize=4)
def _build_nc(cap, n_chunks, chunk):
    nc = bacc.Bacc("TRN2", target_bir_lowering=False, debug=False, num_devices=E)
    xT = nc.dram_tensor("xT", [D_IN, cap], A_DT, kind="ExternalInput")
    w1 = nc.dram_tensor("w1", [P, F_T, D_T, P], W_DT, kind="ExternalInput")
    w2 = nc.dram_tensor("w2", [P, O_T, F_T, P], W_DT, kind="ExternalInput")
    b1t = nc.dram_tensor("b1t", [P, F_T], F32, kind="ExternalInput")
    b2t = nc.dram_tensor("b2t", [P, O_T], F32, kind="ExternalInput")
    yT = nc.dram_tensor("yT", [D_OUT, cap], F32, kind="ExternalOutput")
    with tile.TileContext(nc) as tc:
        _emit(tc, nc, xT, w1, w2, b1t, b2t, yT, cap, n_chunks, chunk)
    nc.compile()
    return nc


def _plan_capacity(max_count):
    cap0 = max(int(max_count), 16)
    n_chunks = max(1, math.ceil(cap0 / 512))
    chunk = math.ceil(cap0 / (n_chunks * 16)) * 16
    return n_chunks * chunk, n_chunks, chunk


def _pack_w1(W1e):
    # w1img[p, i, j, c] = W1e[j*128 + p, i*128 + c]
    return np.ascontiguousarray(
        W1e.reshape(D_T, P, F_T, P).transpose(1, 2, 0, 3).astype(W_NP)
    )


def _pack_w2(W2e):
    # w2img[p, k, i, c] = W2e[i*128 + p, k*128 + c]
    return np.ascontiguousarray(
        W2e.reshape(F_T, P, O_T, P).transpose(1, 2, 0, 3).astype(W_NP)
    )


def _shard(x, groups, W1, b1, W2, b2):
    idx = np.asarray(groups)[:, GROUP_COL].astype(np.int64)
    order = np.argsort(idx, kind="stable")
    counts = np.bincount(idx, minlength=E)
    cap, n_chunks, chunk = _plan_capacity(counts.max())
    offs = np.concatenate([[0], np.cumsum(counts)])

    x = np.asarray(x, dtype=np.float32)
    W1 = np.asarray(W1, dtype=np.float32)
    b1 = np.asarray(b1, dtype=np.float32)
    W2 = np.asarray(W2, dtype=np.float32)
    b2 = np.asarray(b2, dtype=np.float32)

    in_maps, tok_ids = [], []
    for e in range(E):
        ids = order[offs[e] : offs[e + 1]]
        tok_ids.append(ids)
        xT = np.zeros((D_IN, cap), np.float32)
        xT[:, : len(ids)] = x[ids].T
        xT = xT.astype(W_NP)
        in_maps.append(
            {
                "xT": xT,
                "w1": _pack_w1(W1[e]),
                "w2": _pack_w2(W2[e]),
                "b1t": np.ascontiguousarray(b1[e].reshape(F_T, P).T),
                "b2t": np.ascontiguousarray(b2[e].reshape(O_T, P).T),
            }
        )
    return in_maps, tok_ids, counts, cap, n_chunks, chunk


def _run(x, groups, W1, b1, W2, b2, trace=False, **spmd_kwargs):
    in_maps, tok_ids, counts, cap, n_chunks, chunk = _shard(x, groups, W1, b1, W2, b2)
    nc = _build_nc(cap, n_chunks, chunk)
    res = run_bass_kernel_spmd(
        nc, in_maps, core_ids=list(range(E)), trace=trace, **spmd_kwargs
    )
    out = np.zeros((B, D_OUT), np.float32)
    for e in range(E):
        yTe = res.results[e]["yT"]
        out[tok_ids[e]] = yTe[:, : counts[e]].T
    return out, res


def kernel(x, groups, W1, b1, W2, b2):
    out, _ = _run(x, groups, W1, b1, W2, b2)
    return out
